# revision 1
# baseline (speedup 1.0000x reference)
"""Trainium2 Bass kernel for nn_Encoder_Block (B=2,S=2048,D=1024,H=16,FF=4096).

Sharding: 8 cores, core c -> (batch b=c//4, query block q=c%4 of 512 tokens).
Each core recomputes K/V for its whole batch (no cross-core collectives),
everything else is perfectly sharded. Host does transposes and gather.

Device layout: activations kept transposed [feature, token] throughout, so
every matmul in the chain is a natural lhsT/rhs pair with K=128 contraction
chunks and N=512 moving dim. Matmul-fed tensors are fp16 (1 cyc/row like
bf16, but 8x lower rounding noise; fp32 ranges all fit). Attention computes
transposed scores [t, sq]; softmax normalizer rides along the PV matmul as a
ones-column in V (M=65). Masking + 1/sqrt(dh) scaling are folded into the
Exp activation (bias/scale). The softmax reciprocal runs on ACT as
8/nrm = exp(-ln(nrm)+ln 8) (DVE divide is ~8x slower); the 8x keeps fp16
clear of denormals and is folded back in the post-Wo PSUM copy.
"""
import sys, types, os
sys.path.insert(0, "/opt/trn_rl_repo")
import numpy as np
from contextlib import ExitStack

import concourse.bass as bass
import concourse.tile as tile
from concourse import bacc, mybir
from concourse.bass_utils import run_bass_kernel_spmd

B, S, D, H, FF = 2, 2048, 1024, 16, 4096
DH = D // H            # 64
SQ = 512               # query tokens per core
NCORES = 8
NSC = 4                # super-chunks over S (512 keys each)
NTC = 4                # 128-token t-chunks per super-chunk
EPS = 1e-5
MASK_NEG = -60.0       # exp(-60) underflows fp16 => masked keys contribute 0

F32 = mybir.dt.float32
BF = mybir.dt.bfloat16
# bf16, NOT fp16: measured matmul floor is 215ns/512cols (2.38GHz) for bf16
# vs 258ns (1.98GHz) for fp16 on this hardware.
DT = mybir.dt.bfloat16
LN8 = float(np.log(8.0))


def _install_ntff_hook():
    """The image's antenv lacks axon_hooks; shim it so trace=True works."""
    try:
        import antenv.axon_hooks  # noqa
        return
    except ImportError:
        pass
    try:
        from trn_agent_boot.trn_boot import _ntff_profile_via_ctypes
        import antenv
        mod = types.ModuleType("antenv.axon_hooks")
        hook = _ntff_profile_via_ctypes("/opt/axon/libaxon_pjrt.so")
        mod.get_axon_ntff_profile_hook = lambda: hook
        mod.set_axon_ntff_profile_hook = lambda h: None
        sys.modules["antenv.axon_hooks"] = mod
        antenv.axon_hooks = mod
    except Exception:
        pass


def _mm(nc, out, lhsT, rhs, start, stop, tile_position=None):
    nc.tensor.matmul(out, lhsT, rhs,
                     start=start, stop=stop, tile_position=tile_position)


def build_nc():
    nc = bacc.Bacc(trn_type="TRN2", target_bir_lowering=False, debug=False,
                   num_devices=NCORES, dynamic_dma_scratch_size=512)
    AF = mybir.ActivationFunctionType
    OP = mybir.AluOpType

    # ---- DRAM I/O (per-core; program identical across cores) ----
    d_xT = nc.dram_tensor("xT", [D, S], DT, kind="ExternalInput")
    d_xq = nc.dram_tensor("xq", [D, SQ], DT, kind="ExternalInput")
    d_mask = nc.dram_tensor("maskb", [128, S // 128], F32, kind="ExternalInput")
    d_wq = nc.dram_tensor("wq", [D, D], DT, kind="ExternalInput")
    d_wk = nc.dram_tensor("wk", [D, D], DT, kind="ExternalInput")
    d_wv = nc.dram_tensor("wv", [D, D], DT, kind="ExternalInput")
    d_wo = nc.dram_tensor("wo", [D, D], DT, kind="ExternalInput")
    d_aw1 = nc.dram_tensor("aw1", [D, D], DT, kind="ExternalInput")
    d_aw2 = nc.dram_tensor("aw2", [D, D], DT, kind="ExternalInput")
    d_fw1 = nc.dram_tensor("fw1", [D, FF], DT, kind="ExternalInput")
    d_fw2 = nc.dram_tensor("fw2", [FF, D], DT, kind="ExternalInput")
    d_b1 = nc.dram_tensor("b1c", [128, 8], F32, kind="ExternalInput")
    d_g1 = nc.dram_tensor("g1c", [128, 8], F32, kind="ExternalInput")
    d_bb1 = nc.dram_tensor("bb1c", [128, 8], F32, kind="ExternalInput")
    d_fb1 = nc.dram_tensor("fb1c", [128, 32], F32, kind="ExternalInput")
    d_fb2 = nc.dram_tensor("fb2c", [128, 8], F32, kind="ExternalInput")
    d_b2 = nc.dram_tensor("b2c", [128, 8], F32, kind="ExternalInput")
    d_g2 = nc.dram_tensor("g2c", [128, 8], F32, kind="ExternalInput")
    d_bb2 = nc.dram_tensor("bb2c", [128, 8], F32, kind="ExternalInput")
    d_out = nc.dram_tensor("out", [D, SQ], F32, kind="ExternalOutput")

    r_xT = d_xT.ap().rearrange("(c p) s -> p c s", p=128)     # [128, 8, S]
    r_xq = d_xq.ap().rearrange("(c p) s -> p c s", p=128)     # [128, 8, SQ]
    r_wq = d_wq.ap().rearrange("(c p) n -> p c n", p=128)
    r_wk = d_wk.ap().rearrange("(c p) n -> p c n", p=128)
    r_wv = d_wv.ap().rearrange("(c p) n -> p c n", p=128)
    r_wo = d_wo.ap().rearrange("(c p) n -> p c n", p=128)
    r_aw1 = d_aw1.ap().rearrange("(c p) n -> p c n", p=128)
    r_aw2 = d_aw2.ap().rearrange("(c p) n -> p c n", p=128)
    r_fw1 = d_fw1.ap().rearrange("(c p) n -> p c n", p=128)   # [128, 8, FF]
    r_fw2 = d_fw2.ap().rearrange("(c p) n -> p c n", p=128)   # [128, 32, D]
    r_out = d_out.ap().rearrange("(c p) s -> p c s", p=128)

    with tile.TileContext(nc) as tc:
      with ExitStack() as top:
        # one packed const tile (tiles pad to 4KB/partition each otherwise):
        # cols 0:16 maskbias, 16:80 ones, 80 ln8
        const = top.enter_context(tc.tile_pool(name="const", bufs=1))
        cst = const.tile([128, 81], F32, name="cst")
        mask_sb = cst[:, 0:16]
        ones_f = cst[:, 16:80]
        ln8_c = cst[:, 80:81]
        nc.sync.dma_start(mask_sb, d_mask.ap())
        nc.vector.memset(ones_f, 1.0)
        nc.vector.memset(ln8_c, LN8)
        # fp16 constants: selectors for the 1/nrm partition-broadcast matmul,
        # a ones column for LN sums, bf16 ones for the squared sums
        csth = const.tile([128, 257], DT, name="csth")
        sel_e = csth[:, 0:128]
        sel_o = csth[:, 128:256]
        ones_h = csth[:, 256:257]
        nc.vector.memset(csth[:], 0.0)
        # selectors carry the 8x fp16-denormal-guard scale for 1/nrm
        nc.vector.memset(sel_e[0:1, 0:64], 8.0)
        nc.vector.memset(sel_e[32:33, 64:128], 8.0)
        nc.vector.memset(sel_o[64:65, 0:64], 8.0)
        nc.vector.memset(sel_o[96:97, 64:128], 8.0)
        nc.vector.memset(ones_h[:], 1.0)
        ones_b = const.tile([128, 1], BF, name="onesb")
        nc.vector.memset(ones_b[:], 1.0)
        # f32 whose bits are 0x5F3759DF — the fast-rsqrt seed magic
        magic = const.tile([1, SQ], F32, name="magic")
        nc.vector.memset(magic[:], 1.3211836172961054e19)

        def rsqrt_dve(pln, out, x, pref):
            """out = 1/sqrt(x) on DVE via magic seed + 2 Newton steps.

            Avoids the ACT Ln/Exp table loads (1.5us each) that would
            otherwise sit on the LN critical path. x, out: [1, SQ] f32.
            """
            OPa = mybir.AluOpType
            y = pln.tile([1, SQ], F32, name=pref + "rsY")
            t = pln.tile([1, SQ], F32, name=pref + "rsT")
            yu = y.bitcast(mybir.dt.uint32)
            nc.vector.tensor_scalar(yu[:], x.bitcast(mybir.dt.uint32), 1,
                                    None, OPa.logical_shift_right)
            nc.vector.tensor_tensor(yu[:], magic.bitcast(mybir.dt.uint32),
                                    yu[:], OPa.subtract)
            for _ in range(2):
                nc.vector.tensor_mul(t[:], y[:], y[:])
                nc.vector.tensor_mul(t[:], t[:], x)
                nc.vector.tensor_scalar(t[:], t[:], -0.5, 1.5,
                                        OPa.mult, OPa.add)
                nc.vector.tensor_mul(y[:], y[:], t[:])
            nc.vector.tensor_copy(out, y[:])

        # xq stays resident: Q proj input + LN1 residual (DMA issued with the
        # other projection inputs below)
        p_xq = top.enter_context(tc.tile_pool(name="pxq", bufs=1))
        xq_sb = p_xq.tile([128, 8, SQ], DT, name="xqp")
        p_x1 = top.enter_context(tc.tile_pool(name="px1", bufs=1))

        def layernorm_block(st, src_sb, gc, bc, res_sb, dst_sb, pref,
                            out_dma=None, stats=None):
            """dst = LN(src) * g + b + res, all [128, 8, SQ] chunked over D.

            Stats in f32 via ones-matmuls (squares in bf16 for range); the
            apply chain runs in fp16 for 2x DVE rate. The producer loop may
            pass pre-accumulated (ps_s, ps_q) PSUM sums via `stats` so the
            reductions overlap the producing matmuls.
            """
            pln = st.enter_context(tc.tile_pool(name=pref + "ln", bufs=1))
            if stats is None:
                pps = st.enter_context(tc.tile_pool(name=pref + "lps", bufs=1, space="PSUM"))
                sq_sb = pln.tile([128, 8, SQ], BF, name=pref + "sq")
                for d in range(8):
                    nc.vector.tensor_mul(sq_sb[:, d, :], src_sb[:, d, :],
                                         src_sb[:, d, :])
                ps_s = pps.tile([1, SQ], F32, name=pref + "ps_s")
                ps_q = pps.tile([1, SQ], F32, name=pref + "ps_q")
                for d in range(8):
                    _mm(nc, ps_s[:], ones_h, src_sb[:, d, :],
                        start=(d == 0), stop=(d == 7))
                for d in range(8):
                    _mm(nc, ps_q[:], ones_b, sq_sb[:, d, :],
                        start=(d == 0), stop=(d == 7))
            else:
                ps_s, ps_q = stats
            mu = pln.tile([1, SQ], F32, name=pref + "mu")
            nc.scalar.mul(mu[:], ps_s[:], 1.0 / D)
            msq = pln.tile([1, SQ], F32, name=pref + "msq")
            nc.scalar.mul(msq[:], ps_q[:], 1.0 / D)
            var = pln.tile([1, SQ], F32, name=pref + "var")
            nc.vector.tensor_mul(var[:], mu[:], mu[:])
            nc.vector.tensor_sub(var[:], msq[:], var[:])
            nc.vector.tensor_scalar_add(var[:], var[:], EPS)
            rstd = pln.tile([1, SQ], F32, name=pref + "rstd")
            rsqrt_dve(pln, rstd[:], var[:], pref)
            mub = pln.tile([128, SQ], F32, name=pref + "mub")
            rsb = pln.tile([128, SQ], F32, name=pref + "rsb")
            nc.gpsimd.partition_broadcast(mub[:], mu[:])
            nc.gpsimd.partition_broadcast(rsb[:], rstd[:])
            mubh = pln.tile([128, SQ], DT, name=pref + "mubh")
            rsbh = pln.tile([128, SQ], DT, name=pref + "rsbh")
            nc.vector.tensor_copy(mubh[:], mub[:])
            nc.vector.tensor_copy(rsbh[:], rsb[:])
            tmp = pln.tile([128, 8, SQ], DT, name=pref + "tmp")
            for d in range(8):
                nc.vector.tensor_sub(tmp[:, d, :], src_sb[:, d, :], mubh[:])
                nc.vector.tensor_mul(tmp[:, d, :], tmp[:, d, :], rsbh[:])
                nc.vector.tensor_scalar(tmp[:, d, :], tmp[:, d, :],
                                        gc[:, d:d + 1], bc[:, d:d + 1],
                                        OP.mult, OP.add)
                nc.vector.tensor_add(dst_sb[:, d, :], tmp[:, d, :],
                                     res_sb[:, d, :])
                if out_dma is not None:
                    nc.sync.dma_start(out_dma[:, d, :], dst_sb[:, d, :])

        # ============ Stages 1-3 share one scope: attention weights are
        # ============ tag-reused for the post-attention weights so their
        # ============ DMAs overlap the attention phase.
        with ExitStack() as s13:
            p_acc = s13.enter_context(tc.tile_pool(name="acc", bufs=1))
            acc = p_acc.tile([128, 8, SQ], DT, name="acc")
            # softmax denominators at partition 32*(h%4), free idx h//4;
            # init 1.0 so unused rows stay finite through reciprocal+selector
            nrm = p_acc.tile([128, 4, SQ], F32, name="nrm")
            nc.vector.memset(nrm[:], 1.0)

            pwkv = s13.enter_context(tc.tile_pool(name="pwkv", bufs=1))
            wk_sb = pwkv.tile([128, 8, D], DT, name="wk", tag="wk")
            wv_sb = pwkv.tile([128, 8, D], DT, name="wv", tag="wv")
            pxsc = s13.enter_context(tc.tile_pool(name="pxsc", bufs=1))
            xs0 = pxsc.tile([128, 8, 512], DT, name="xsc", tag="xsc")

            with ExitStack() as A:
                p_qT = A.enter_context(tc.tile_pool(name="qT", bufs=1))
                qT = p_qT.tile([128, 8, SQ], DT, name="qT")

                # ---- Stage 1a: Q^T projection, d-outer so the first matmul
                # ---- needs only one wq/xq chunk pair (fast start) ----
                with ExitStack() as st:
                    pw = st.enter_context(tc.tile_pool(name="pwq", bufs=1))
                    pp = st.enter_context(tc.tile_pool(name="ppq", bufs=1, space="PSUM"))
                    wq_sb = pw.tile([128, 8, D], DT, name="wq")
                    # DMA priority: Q-proj inputs, then K-proj (wk + first x
                    # super-chunk), then wv — matches first-use order so the
                    # PE starts ~1.5us in and never starves
                    for d in range(8):
                        nc.sync.dma_start(wq_sb[:, d, :], r_wq[:, d, :])
                        nc.sync.dma_start(xq_sb[:, d, :], r_xq[:, d, :])
                    for d in range(8):
                        nc.sync.dma_start(wk_sb[:, d, :], r_wk[:, d, :])
                        nc.sync.dma_start(xs0[:, d, :], r_xT[:, d, 0:512])
                    for d in range(8):
                        nc.sync.dma_start(wv_sb[:, d, :], r_wv[:, d, :])
                    psq = [pp.tile([128, SQ], F32, name=f"psq{p}")
                           for p in range(8)]
                    for d in range(8):
                        for p in range(8):
                            _mm(nc, psq[p][:], wq_sb[:, d, p * 128:(p + 1) * 128],
                                xq_sb[:, d, :], start=(d == 0), stop=(d == 7))
                    for p in range(8):
                        nc.scalar.copy(qT[:, p, :], psq[p][:])

                # ---- Stage 1b+2: K/V proj + attention, flash over 4 sc ----
                pkv = A.enter_context(tc.tile_pool(name="pkv", bufs=2))
                pexp = A.enter_context(tc.tile_pool(name="pexp", bufs=4))
                aps = A.enter_context(ExitStack())
                # PSUM: scores 2x2 banks + PV 1x2 banks + K/V-proj 2x1 bank
                psc = aps.enter_context(tc.tile_pool(name="psc", bufs=2, space="PSUM"))
                ppv = aps.enter_context(tc.tile_pool(name="ppv", bufs=1, space="PSUM"))
                ppj = aps.enter_context(tc.tile_pool(name="ppj", bufs=2, space="PSUM"))

                for sc in range(NSC):
                    t0 = sc * 512
                    if sc == 0:
                        xs = xs0
                    else:
                        xs = pxsc.tile([128, 8, 512], DT, name="xsc", tag="xsc")
                        for d in range(8):
                            nc.sync.dma_start(xs[:, d, :],
                                              r_xT[:, d, t0:t0 + 512])

                    kT = pkv.tile([128, 8, 512], DT, name="kT")
                    for p in range(8):
                        ps = ppj.tile([128, SQ], F32, name="pskv")
                        for d in range(8):
                            _mm(nc, ps[:], wk_sb[:, d, p * 128:(p + 1) * 128],
                                xs[:, d, :], start=(d == 0), stop=(d == 7))
                        nc.vector.tensor_copy(kT[:, p, :], ps[:])

                    vt = pkv.tile([128, NTC, 16, 65], DT, name="vt")
                    nc.vector.tensor_copy(
                        vt[:, :, :, 64:65],
                        ones_f.rearrange("p (a b c) -> p a b c", a=NTC, b=16))
                    for i in range(NTC):
                        for nb in range(2):
                            ps = ppj.tile([128, SQ], F32, name="pskv")
                            for d in range(8):
                                _mm(nc, ps[:], xs[:, d, i * 128:(i + 1) * 128],
                                    wv_sb[:, d, nb * 512:(nb + 1) * 512],
                                    start=(d == 0), stop=(d == 7))
                            # V evacuation on ACT: the transposed PSUM read
                            # is DVE's slowest op here and competes with the
                            # PV-accumulator drains that gate each head-pair
                            nc.scalar.copy(
                                vt[:, i, nb * 8:(nb + 1) * 8, 0:64],
                                ps.rearrange("p (h e) -> p h e", e=64))

                    for p in range(8):
                        h0, h1 = 2 * p, 2 * p + 1
                        pva = ppv.tile([128, 2, SQ], F32, name="pva")
                        for i in range(NTC):
                            tci = sc * NTC + i
                            s01 = psc.tile([128, 2, SQ], F32, name="s01")
                            _mm(nc, s01[:, 0, :],
                                kT[0:64, p, i * 128:(i + 1) * 128],
                                qT[0:64, p, :], start=True, stop=True,
                                tile_position=(0, 0))
                            _mm(nc, s01[:, 1, :],
                                kT[64:128, p, i * 128:(i + 1) * 128],
                                qT[64:128, p, :], start=True, stop=True,
                                tile_position=(64, 0))
                            e01 = pexp.tile([128, 2, SQ], DT, name="e01")
                            nc.scalar.activation(e01[:], s01[:], AF.Exp,
                                                 bias=mask_sb[:, tci:tci + 1],
                                                 scale=0.125)
                            _mm(nc, pva[0:65, 0, :], vt[:, i, h0, :], e01[:, 0, :],
                                start=(i == 0), stop=(i == NTC - 1))
                            _mm(nc, pva[0:65, 1, :], vt[:, i, h1, :], e01[:, 1, :],
                                start=(i == 0), stop=(i == NTC - 1))
                        a0, c0 = 32 * (h0 % 4), h0 // 4
                        a1, c1 = 32 * (h1 % 4), h1 // 4
                        if sc == 0:
                            nc.vector.tensor_copy(acc[0:64, p, :], pva[0:64, 0, :])
                            nc.vector.tensor_copy(acc[64:128, p, :], pva[0:64, 1, :])
                            nc.vector.tensor_copy(nrm[a0:a0 + 1, c0, :], pva[64:65, 0, :])
                            nc.vector.tensor_copy(nrm[a1:a1 + 1, c1, :], pva[64:65, 1, :])
                        else:
                            nc.vector.tensor_add(acc[0:64, p, :],
                                                 acc[0:64, p, :], pva[0:64, 0, :])
                            nc.vector.tensor_add(acc[64:128, p, :],
                                                 acc[64:128, p, :], pva[0:64, 1, :])
                            nc.vector.tensor_add(nrm[a0:a0 + 1, c0, :],
                                                 nrm[a0:a0 + 1, c0, :], pva[64:65, 0, :])
                            nc.vector.tensor_add(nrm[a1:a1 + 1, c1, :],
                                                 nrm[a1:a1 + 1, c1, :], pva[64:65, 1, :])

                # normalize: acc *= 8/nrm via selector-matmul bcast (the 8x —
                # an fp16 denormal guard — rides in the selector constants
                # and is folded back in the Wo-PSUM copy). The reciprocal is
                # a custom-DVE approx op (full divide is ~5x slower, ACT
                # Ln/Exp would thrash activation tables). Done per nrm
                # column: column c is final right after sc3's p=2c+1
                # iteration, so normalization overlaps the attention tail —
                # the broadcast matmuls reuse ppj's banks for the same
                # reason.
                pnr = A.enter_context(tc.tile_pool(name="pnr", bufs=1))
                rcp = pnr.tile([128, 4, SQ], F32, name="rcp")
                nrm8 = pnr.tile([128, 4, SQ], DT, name="nrm8")
                for c in range(4):
                    nc.vector.reciprocal_approx_fast(rcp[:, c, :], nrm[:, c, :])
                    nc.vector.tensor_copy(nrm8[:, c, :], rcp[:, c, :])
                    for p in (2 * c, 2 * c + 1):
                        sel = sel_e if p % 2 == 0 else sel_o
                        ps_rb = ppj.tile([128, SQ], F32, name="pskv")
                        nc.tensor.matmul(ps_rb[:], sel, nrm8[:, c, :],
                                         start=True, stop=True)
                        nc.vector.tensor_mul(acc[:, p, :], acc[:, p, :],
                                             ps_rb[:])
                aps.close()

            # ---- Stage 3: Wo + add1 + LN1 + residual (weights tag-reuse
            # ---- wk/wv/xsc slots so the DMAs run during attention) ----
            with ExitStack() as st:
                wo_sb = pwkv.tile([128, 8, D], DT, name="wo", tag="wk")
                for do in range(8):
                    nc.sync.dma_start(wo_sb[:, :, do * 128:(do + 1) * 128],
                                      r_wo[:, :, do * 128:(do + 1) * 128])
                aw1_sb = pwkv.tile([128, 8, D], DT, name="aw1", tag="wv")
                for do in range(8):
                    nc.sync.dma_start(aw1_sb[:, :, do * 128:(do + 1) * 128],
                                      r_aw1[:, :, do * 128:(do + 1) * 128])
                pw = st.enter_context(tc.tile_pool(name="pw3", bufs=1))
                b1_sb = pw.tile([128, 8], F32, name="b1")
                nc.sync.dma_start(b1_sb[:], d_b1.ap())
                g1_sb = pw.tile([128, 8], F32, name="g1")
                nc.sync.dma_start(g1_sb[:], d_g1.ap())
                bb1_sb = pw.tile([128, 8], F32, name="bb1")
                nc.sync.dma_start(bb1_sb[:], d_bb1.ap())

                x1 = p_x1.tile([128, 8, SQ], DT, name="x1")
                pao = st.enter_context(tc.tile_pool(name="pao", bufs=1))
                ao = pao.tile([128, 8, SQ], DT, name="ao")
                pp = st.enter_context(tc.tile_pool(name="pp3", bufs=2, space="PSUM"))
                # psum = 8*(attout@wo); the 1/8 rides the evacuation copy
                for do in range(8):
                    ps = pp.tile([128, SQ], F32, name="ps3a")
                    for d in range(8):
                        _mm(nc, ps[:], wo_sb[:, d, do * 128:(do + 1) * 128],
                            acc[:, d, :], start=(d == 0), stop=(d == 7))
                    nc.scalar.mul(ao[:, do, :], ps[:], 1.0 / 8.0)
                l1 = pao.tile([128, 8, SQ], DT, name="l1")
                sq1 = pao.tile([128, 8, SQ], BF, name="sq1")
                pst = st.enter_context(tc.tile_pool(name="pst3", bufs=1, space="PSUM"))
                ps_s = pst.tile([1, SQ], F32, name="ps_s3")
                ps_q = pst.tile([1, SQ], F32, name="ps_q3")
                # aw1 runs d-outer in two 4-bank half-passes: its d-th matmul
                # group needs only ao[:, d, :], so it trails the Wo do-loop
                # by one chunk instead of serializing after it.
                paw = st.enter_context(tc.tile_pool(name="paw1", bufs=1, space="PSUM"))
                psl = [paw.tile([128, SQ], F32, name=f"aw1p{j}")
                       for j in range(4)]
                for half in range(2):
                    for d in range(8):
                        for j in range(4):
                            do = half * 4 + j
                            _mm(nc, psl[j][:],
                                aw1_sb[:, d, do * 128:(do + 1) * 128],
                                ao[:, d, :], start=(d == 0), stop=(d == 7))
                    for j in range(4):
                        do = half * 4 + j
                        nc.vector.tensor_scalar(l1[:, do, :], psl[j][:],
                                                b1_sb[:, do:do + 1], None,
                                                OP.add)
                        nc.vector.tensor_mul(sq1[:, do, :], l1[:, do, :],
                                             l1[:, do, :])
                        _mm(nc, ps_s[:], ones_h, l1[:, do, :],
                            start=(do == 0), stop=(do == 7))
                        _mm(nc, ps_q[:], ones_b, sq1[:, do, :],
                            start=(do == 0), stop=(do == 7))
                layernorm_block(st, l1, g1_sb, bb1_sb, xq_sb, x1, "a",
                                stats=(ps_s, ps_q))

        # ================= Stage 4: FFN + add2 + LN2 + residual =================
        with ExitStack() as st:
            pff = st.enter_context(tc.tile_pool(name="pff", bufs=1))
            ff = pff.tile([128, 8, SQ], DT, name="ff")
            aw2_sb = pff.tile([128, 8, D], DT, name="aw2")
            for do in range(8):
                nc.sync.dma_start(aw2_sb[:, :, do * 128:(do + 1) * 128],
                                  r_aw2[:, :, do * 128:(do + 1) * 128])
            # aw2's accumulators live alongside the FFN's PSUM (4+2+2 banks)
            # so its d-outer groups can interleave with the fw2 stream: the
            # d-th group needs only ff[:, d, :], not the whole FFN output.
            l2 = pff.tile([128, 8, SQ], DT, name="l2")
            sq2 = pff.tile([128, 8, SQ], BF, name="sq2")
            b2_sb = pff.tile([128, 8], F32, name="b2")
            nc.sync.dma_start(b2_sb[:], d_b2.ap())
            paw2 = st.enter_context(tc.tile_pool(name="paw2", bufs=1, space="PSUM"))
            psl2 = [paw2.tile([128, SQ], F32, name=f"aw2p{j}")
                    for j in range(4)]
            pst4 = st.enter_context(tc.tile_pool(name="pst4", bufs=1, space="PSUM"))
            ps_s4 = pst4.tile([1, SQ], F32, name="ps_s4")
            ps_q4 = pst4.tile([1, SQ], F32, name="ps_q4")
            with ExitStack() as st4a:
                ph = st4a.enter_context(tc.tile_pool(name="ph", bufs=1))
                h_sb = ph.tile([128, 32, SQ], DT, name="h")
                pwc = st4a.enter_context(tc.tile_pool(name="pwc", bufs=6))
                pwc2 = st4a.enter_context(tc.tile_pool(name="pwc2", bufs=2))
                fb1_sb = ph.tile([128, 32], F32, name="fb1")
                nc.sync.dma_start(fb1_sb[:], d_fb1.ap())
                fb2_sb = ph.tile([128, 8], F32, name="fb2")
                nc.sync.dma_start(fb2_sb[:], d_fb2.ap())
                pp = st4a.enter_context(tc.tile_pool(name="pp4", bufs=2, space="PSUM"))

                for f in range(32):
                    w1t = pwc.tile([128, 8, 128], DT, name="w1c")
                    nc.sync.dma_start(w1t[:], r_fw1[:, :, f * 128:(f + 1) * 128])
                    ps = pp.tile([128, SQ], F32, name="ps4a")
                    for d in range(8):
                        _mm(nc, ps[:], w1t[:, d, :], x1[:, d, :],
                            start=(d == 0), stop=(d == 7))
                    nc.vector.tensor_scalar(h_sb[:, f, :], ps[:],
                                            fb1_sb[:, f:f + 1], 0.0,
                                            OP.add, OP.max)

                for do in range(8):
                    w2t = pwc2.tile([128, 32, 128], DT, name="w2c")
                    nc.sync.dma_start(w2t[:], r_fw2[:, :, do * 128:(do + 1) * 128])
                    ps = pp.tile([128, SQ], F32, name="ps4a")
                    for f in range(32):
                        _mm(nc, ps[:], w2t[:, f, :], h_sb[:, f, :],
                            start=(f == 0), stop=(f == 31))
                    nc.vector.tensor_scalar(ff[:, do, :], ps[:],
                                            fb2_sb[:, do:do + 1], None, OP.add)

            for half in range(2):
                for d in range(8):
                    for j in range(4):
                        do = half * 4 + j
                        _mm(nc, psl2[j][:],
                            aw2_sb[:, d, do * 128:(do + 1) * 128],
                            ff[:, d, :], start=(d == 0), stop=(d == 7))
                for j in range(4):
                    do = half * 4 + j
                    nc.vector.tensor_scalar(l2[:, do, :], psl2[j][:],
                                            b2_sb[:, do:do + 1], None, OP.add)
                    nc.vector.tensor_mul(sq2[:, do, :], l2[:, do, :],
                                         l2[:, do, :])
                    _mm(nc, ps_s4[:], ones_h, l2[:, do, :],
                        start=(do == 0), stop=(do == 7))
                    _mm(nc, ps_q4[:], ones_b, sq2[:, do, :],
                        start=(do == 0), stop=(do == 7))

            with ExitStack() as st4b:
                pw = st4b.enter_context(tc.tile_pool(name="pw4", bufs=1))
                g2_sb = pw.tile([128, 8], F32, name="g2")
                nc.sync.dma_start(g2_sb[:], d_g2.ap())
                bb2_sb = pw.tile([128, 8], F32, name="bb2")
                nc.sync.dma_start(bb2_sb[:], d_bb2.ap())
                outp = pw.tile([128, 8, SQ], F32, name="outp")
                layernorm_block(st4b, l2, g2_sb, bb2_sb, x1, outp, "b",
                                out_dma=r_out, stats=(ps_s4, ps_q4))

    nc.compile()
    return nc


_NC = None


def _get_nc():
    global _NC
    if _NC is None:
        _NC = build_nc()
    return _NC


def _prep_inputs(inputs):
    """Host-side shard prep: per-core input dicts."""
    np_dt = mybir.dt.np(DT)
    x = np.asarray(inputs["batch_x"], np.float32)       # [B, S, D]
    lens = np.asarray(inputs["len_chair"], np.int64)

    def cvt(a):
        return np.ascontiguousarray(np.asarray(a, np.float32).astype(np_dt))

    wq = np.asarray(inputs["Wq"], np.float32).transpose(1, 0, 2).reshape(D, D)
    wk = np.asarray(inputs["Wk"], np.float32).transpose(1, 0, 2).reshape(D, D)
    wv = np.asarray(inputs["Wv"], np.float32).transpose(1, 0, 2).reshape(D, D)
    com = {
        "wq": cvt(wq), "wk": cvt(wk), "wv": cvt(wv),
        "wo": cvt(inputs["Wo"]), "aw1": cvt(inputs["add1_w"]),
        "aw2": cvt(inputs["add2_w"]), "fw1": cvt(inputs["ff_w1"]),
        "fw2": cvt(inputs["ff_w2"]),
        "b1c": _chunk(inputs["add1_b"]), "g1c": _chunk(inputs["ln1_g"]),
        "bb1c": _chunk(inputs["ln1_b"]), "fb1c": _chunk(inputs["ff_b1"]),
        "fb2c": _chunk(inputs["ff_b2"]), "b2c": _chunk(inputs["add2_b"]),
        "g2c": _chunk(inputs["ln2_g"]), "bb2c": _chunk(inputs["ln2_b"]),
    }
    xT = [cvt(x[b].T) for b in range(B)]                # [D, S]
    masks = []
    for b in range(B):
        m = np.where(np.arange(S) >= lens[b], np.float32(MASK_NEG),
                     np.float32(0.0)).astype(np.float32)
        masks.append(np.ascontiguousarray(m.reshape(S // 128, 128).T))
    in_maps = []
    for c in range(NCORES):
        b, q = c // 4, c % 4
        m = dict(com)
        m["xT"] = xT[b]
        m["xq"] = np.ascontiguousarray(xT[b][:, q * SQ:(q + 1) * SQ])
        m["maskb"] = masks[b]
        in_maps.append(m)
    return in_maps


def _chunk(v):
    v = np.asarray(v, np.float32)
    return np.ascontiguousarray(v.reshape(-1, 128).T)


def kernel(trace=False, **inputs):
    _install_ntff_hook()
    nc = _get_nc()
    in_maps = _prep_inputs(inputs)
    res = run_bass_kernel_spmd(nc, in_maps, core_ids=list(range(NCORES)),
                               trace=trace)
    out = np.empty((B, S, D), np.float32)
    for c in range(NCORES):
        b, q = c // 4, c % 4
        out[b, q * SQ:(q + 1) * SQ, :] = res.results[c]["out"].T
    kernel.last_exec_time_ns = res.exec_time_ns
    return out

